# revision 14
# baseline (speedup 1.0000x reference)
# Trainium2 Bass kernel for nn_DecoderLayer (dense transformer decoder layer).
#
# Sharding: 8 cores = 2 batch groups (data-parallel over B=2) x 4-way
# sequence-shard of the 2048 query rows (512 rows/core).  Each core computes
# K/V projections for its batch's full sequence (redundant within the group,
# zero communication), attention for its own query rows over all 16 heads,
# then the output projection / layernorms / FFN for its own rows.
#
# Layout: activations are kept TRANSPOSED on-chip ([D, rows]; D on SBUF
# partitions) so that every linear layer is `out_T = W.T @ x_T` with the
# natural row-major weight as the stationary operand and no transposes
# anywhere.  Matmuls run as float32r (full-rate fp32 mode).  Softmax is
# max-free (scores are O(1) here) and the denominator comes for free from a
# ones-augmented V column.  LayerNorm reductions over the partition axis use
# ones-vector matmuls on the PE.
import numpy as np

import concourse.bacc as bacc
import concourse.bass as bass
import concourse.mybir as mybir
import concourse.tile as tile
from concourse.bass import ds
from concourse.bass_utils import run_bass_kernel_spmd

B, S, D, H, DK, F = 2, 2048, 1024, 16, 64, 4096
P = 128
NCORES = 8
GP = 4            # cores per batch group
R = S // GP       # query rows per core (512)
DC = D // P       # 8 chunks of D
FC = F // P       # 32 chunks of F
SB = S // P       # 16 key blocks of 128
SQW = 256         # seq chunk width for kv projection rhs
NSQ = S // SQW    # 8
EPS = 1e-5
F32 = mybir.dt.float32
F32R = mybir.dt.float32r
AF = mybir.ActivationFunctionType
ALU = mybir.AluOpType

WDD = ["wq1", "wk1", "wv1", "wo1", "wq2", "wk2", "wv2", "wo2"]
BDD = ["bq1", "bk1", "bv1", "bo1", "bq2", "bk2", "bv2", "bo2"]
LNP = ["g1", "be1", "g2", "be2", "g3", "be3"]


def _bcast_ap(ap, parts):
    # [n] DRAM vector -> [parts, n] partition-broadcast AP
    return bass.AP(tensor=ap.tensor, offset=ap.offset, ap=[[0, parts]] + list(ap.ap))


def _build_body(tc, io):
    nc = tc.nc

    psA = tc.alloc_tile_pool(name="psA", bufs=2, space="PSUM")  # projections
    psS = tc.alloc_tile_pool(name="psS", bufs=2, space="PSUM")  # scores
    psV = tc.alloc_tile_pool(name="psV", bufs=2, space="PSUM")  # attn*V
    psR = tc.alloc_tile_pool(name="psR", bufs=2, space="PSUM")  # LN reductions
    dram = tc.alloc_tile_pool(name="dram", bufs=2, space="DRAM")
    consts = tc.alloc_tile_pool(name="consts", bufs=1)
    persist = tc.alloc_tile_pool(name="persist", bufs=1)

    # ---- constants ------------------------------------------------------
    ones_f = consts.tile([P, P], F32)
    nc.vector.memset(ones_f, 1.0)
    ones = consts.tile([P, 1], F32R)
    nc.vector.tensor_copy(out=ones, in_=ones_f[:, 0:1])
    ones_row = consts.tile([1, P], F32R)
    nc.vector.tensor_copy(out=ones_row, in_=ones_f[0:1, :])
    ones_col = consts.tile([P, SB, 1], F32)
    nc.vector.memset(ones_col, 1.0)
    bsb = {}
    for n in BDD + LNP + ["bf2"]:
        t = consts.tile([P, DC], F32, name=f"c_{n}", tag=f"c_{n}")
        nc.sync.dma_start(out=t, in_=io[n].rearrange("(c p) -> p c", p=P))
        bsb[n] = t
    bf1_sb = consts.tile([P, FC], F32)
    nc.sync.dma_start(out=bf1_sb, in_=io["bf1"].rearrange("(c p) -> p c", p=P))
    bvbc = {}
    for n in ["bv1", "bv2"]:
        t = consts.tile([P, D], F32, name=f"bc_{n}", tag=f"bc_{n}")
        nc.sync.dma_start(out=t, in_=_bcast_ap(io[n], P))
        bvbc[n] = t

    # persistent residual-chain tiles ([P, DC, R], 2 live at a time)
    def res_tile(name):
        return persist.tile([P, DC, R], F32R, name=name, tag="res", bufs=2)

    def layernorm(pool, x_sb, g_sb, be_sb, out_chunk_fn):
        """LN over the partition(x chunk) axis of x_sb [P, DC, R].
        out_chunk_fn(c, src_ap) consumes the normalized chunk."""
        ps_sum = psR.tile([1, R], F32, tag="red")
        ps_sq = psR.tile([1, R], F32, tag="red")
        for c in range(DC):
            sqc = pool.tile([P, R], F32R, name="sqc", tag="sqc", bufs=2)
            nc.vector.tensor_mul(sqc, x_sb[:, c, :], x_sb[:, c, :])
            nc.tensor.matmul(ps_sum, (ones), (x_sb[:, c, :]),
                             start=(c == 0), stop=(c == DC - 1))
            nc.tensor.matmul(ps_sq, (ones), (sqc),
                             start=(c == 0), stop=(c == DC - 1))
        mean = pool.tile([1, R], F32R, name="mean", tag="st", bufs=8)
        nc.scalar.mul(mean, ps_sum, 1.0 / D)
        msq = pool.tile([1, R], F32, name="msq", tag="st", bufs=8)
        nc.vector.tensor_mul(msq, mean, ps_sum)          # sum^2 / D
        varn = pool.tile([1, R], F32, name="varn", tag="st", bufs=8)
        nc.vector.tensor_sub(varn, ps_sq, msq)
        sd = pool.tile([1, R], F32, name="sd", tag="st", bufs=8)
        nc.scalar.activation(sd, varn, AF.Sqrt, scale=1.0 / (D - 1))
        nc.vector.tensor_scalar_add(sd, sd, EPS)
        rr = pool.tile([1, R], F32R, name="rr", tag="st", bufs=8)
        with nc.allow_low_precision(reason="f32r is same-width as f32"):
            nc.vector.reciprocal(rr, sd)
        # broadcast mean/rstd across partitions via K=1 outer-product matmuls
        mb = psA.tile([P, R], F32, name="mb", tag="proj")
        nc.tensor.matmul(mb, (ones_row), (mean), start=True, stop=True)
        rb = psA.tile([P, R], F32, name="rb", tag="proj")
        nc.tensor.matmul(rb, (ones_row), (rr), start=True, stop=True)
        for c in range(DC):
            t = pool.tile([P, R], F32, name="lnt", tag="sqc", bufs=2)
            nc.vector.tensor_sub(t, x_sb[:, c, :], mb)
            nc.vector.tensor_mul(t, t, rb)
            out_chunk_fn(c, t, g_sb[:, c:c + 1], be_sb[:, c:c + 1])

    def attn_block(blk, kvT, qres_sb, wq, wk, wv, wo, bq, bk, bv_bc, bo,
                   use_mask, g_sb, be_sb):
        """One attention sublayer. Returns out_sb (post-LN, [P, DC, R])."""
        kTd = dram.tile([D, S], F32R, name=f"kTd{blk}", tag="kTd")
        vd = dram.tile([S, D], F32R, name=f"vd{blk}", tag="vd")
        blkpool = tc.alloc_tile_pool(name=f"blk{blk}", bufs=1)
        qT = blkpool.tile([P, DC, R], F32R, name="qT", tag="qT")
        attT = blkpool.tile([P, DC, R], F32R, name="attT", tag="attT")

        # ---- K^T and V projections over the full kv sequence ------------
        pp = tc.alloc_tile_pool(name=f"proj{blk}", bufs=1)
        wk_sb = pp.tile([P, DC, D], F32R, name="wk_sb", tag="wdd", bufs=2)
        nc.sync.dma_start(out=wk_sb, in_=wk.bitcast(F32R).rearrange("(c p) n -> p c n", p=P))
        wv_sb = pp.tile([P, DC, D], F32R, name="wv_sb", tag="wdd", bufs=2)
        nc.sync.dma_start(out=wv_sb, in_=wv.bitcast(F32R).rearrange("(c p) n -> p c n", p=P))
        for sq in range(NSQ):
            ytr = pp.tile([P, DC, SQW], F32R, name="ytr", tag="ytr", bufs=2)
            nc.sync.dma_start(
                out=ytr, in_=kvT.bitcast(F32R)[:, ds(SQW * sq, SQW)].rearrange(
                    "(c p) s -> p c s", p=P))
            # kT chunk: [dout 128, SQW]
            for do in range(DC):
                psf = psA.tile([P, 512], F32, name="psf", tag="proj")
                ps = psf[:, :SQW]
                for kc in range(DC):
                    nc.tensor.matmul(ps, (wk_sb[:, kc, ds(P * do, P)]),
                                     (ytr[:, kc, :]),
                                     start=(kc == 0), stop=(kc == DC - 1))
                stg = pp.tile([P, SQW], F32R, name="kstg", tag="stg", bufs=3)
                nc.scalar.add(stg, ps, bk[:, do:do + 1])
                nc.sync.dma_start(out=kTd[ds(P * do, P), ds(SQW * sq, SQW)],
                                  in_=stg)
            # v rows: [rows 128, dout 512]
            for rbl in range(SQW // P):
                for dn in range(2):
                    ps = psA.tile([P, 512], F32, tag="proj")
                    for kc in range(DC):
                        nc.tensor.matmul(ps, (ytr[:, kc, ds(P * rbl, P)]),
                                         (wv_sb[:, kc, ds(512 * dn, 512)]),
                                         start=(kc == 0), stop=(kc == DC - 1))
                    stg = pp.tile([P, 512], F32R, name="vstg", tag="vstg",
                                  bufs=3)
                    nc.vector.tensor_add(stg, ps, bv_bc[:, ds(512 * dn, 512)])
                    nc.sync.dma_start(
                        out=vd[ds(SQW * sq + P * rbl, P), ds(512 * dn, 512)],
                        in_=stg)
        # ---- Q projection (own rows) -------------------------------------
        for do in range(DC):
            wqs = pp.tile([P, DC, P], F32R, name="wqs", tag="wcol", bufs=3)
            nc.sync.dma_start(
                out=wqs, in_=wq.bitcast(F32R)[:, ds(P * do, P)].rearrange(
                    "(c p) n -> p c n", p=P))
            ps = psA.tile([P, R], F32, tag="proj")
            for kc in range(DC):
                nc.tensor.matmul(ps, (wqs[:, kc, :]), (qres_sb[:, kc, :]),
                                 start=(kc == 0), stop=(kc == DC - 1))
            nc.scalar.add(qT[:, do, :], ps, bq[:, do:do + 1])
        pp.release()

        # ---- attention over head pairs -----------------------------------
        ap_ = tc.alloc_tile_pool(name=f"attn{blk}", bufs=1)
        if use_mask:
            mask_sb = ap_.tile([P, SB, R], F32, name="mask_sb", tag="mask")
            nc.sync.dma_start(
                out=mask_sb,
                in_=io["mask"].rearrange("(kb p) q -> p kb q", p=P))
        for t in range(H // 2):
            khp = ap_.tile([P, S], F32R, name="khp", tag="khp", bufs=2)
            nc.sync.dma_start(out=khp, in_=kTd[ds(P * t, P), :])
            vab = []
            for a in range(2):
                va = ap_.tile([P, SB, DK + 1], F32R, name=f"va{a}",
                              tag="vaug", bufs=4)
                nc.sync.dma_start(
                    out=va[:, :, 0:DK],
                    in_=vd[:, ds(P * t + DK * a, DK)].rearrange(
                        "(kb p) d -> p kb d", p=P))
                nc.vector.tensor_copy(out=va[:, :, DK:DK + 1], in_=ones_col)
                vab.append(va)
            pv = [psV.tile([DK + 1, R], F32, tag="av", name=f"pv{a}")
                  for a in range(2)]
            for kb in range(SB):
                for a in range(2):
                    pss = psS.tile([P, R], F32, tag="sc")
                    nc.tensor.matmul(
                        pss,
                        (khp[ds(DK * a, DK), ds(P * kb, P)]),
                        (qT[ds(DK * a, DK), t, :]),
                        start=True, stop=True,
                        tile_position=(DK * a, 0))
                    ex = ap_.tile([P, R], F32R, name="ex", tag="exp", bufs=4)
                    nc.scalar.activation(ex, pss, AF.Exp, scale=0.125)
                    if use_mask:
                        nc.vector.tensor_mul(ex, ex, mask_sb[:, kb, :])
                    nc.tensor.matmul(pv[a], (vab[a][:, kb, :]), (ex),
                                     start=(kb == 0), stop=(kb == SB - 1))
            for a in range(2):
                rc = ap_.tile([1, R], F32R, name="rc", tag="rc", bufs=2)
                with nc.allow_low_precision(reason="f32r is same-width as f32"):
                    nc.vector.reciprocal(rc, pv[a][DK:DK + 1, :])
                rcb = psA.tile([P, R], F32, name="rcb", tag="proj")
                nc.tensor.matmul(rcb[:DK], (ones_row[:, :DK]), (rc),
                                 start=True, stop=True)
                rcs = ap_.tile([DK, R], F32R, name="rcs", tag="rcs", bufs=2)
                nc.vector.tensor_copy(out=rcs, in_=rcb[:DK])
                nc.vector.tensor_mul(attT[ds(DK * a, DK), t, :],
                                     pv[a][0:DK, :], rcs)
        ap_.release()

        # ---- output projection + residual + LN ---------------------------
        op = tc.alloc_tile_pool(name=f"oproj{blk}", bufs=1)
        res_sb = res_tile(f"res{blk}")
        for do in range(DC):
            wos = op.tile([P, DC, P], F32R, name="wos", tag="wcol", bufs=3)
            nc.sync.dma_start(
                out=wos, in_=wo.bitcast(F32R)[:, ds(P * do, P)].rearrange(
                    "(c p) n -> p c n", p=P))
            ps = psA.tile([P, R], F32, tag="proj")
            for kc in range(DC):
                nc.tensor.matmul(ps, (wos[:, kc, :]), (attT[:, kc, :]),
                                 start=(kc == 0), stop=(kc == DC - 1))
            nc.scalar.add(res_sb[:, do, :], ps, bo[:, do:do + 1])
            nc.vector.tensor_add(res_sb[:, do, :], res_sb[:, do, :],
                                 qres_sb[:, do, :])
        # LN in-place into res_sb (stats computed first, then rewrite)
        def _emit(c, t, g, be):
            nc.vector.tensor_scalar(res_sb[:, c, :], t, g, be,
                                    op0=ALU.mult, op1=ALU.add)
        layernorm(op, res_sb, g_sb, be_sb, _emit)
        op.release()
        blkpool.release()
        return res_sb

    # ---- main flow -------------------------------------------------------
    src1 = persist.tile([P, DC, R], F32R, name="src1", tag="src1")
    nc.sync.dma_start(out=src1, in_=io["yTo"].bitcast(F32R).rearrange(
        "(c p) q -> p c q", p=P))
    out1 = attn_block(1, io["yT"], src1, io["wq1"], io["wk1"], io["wv1"],
                      io["wo1"], bsb["bq1"], bsb["bk1"], bvbc["bv1"],
                      bsb["bo1"], True, bsb["g1"], bsb["be1"])
    out2 = attn_block(2, io["xT"], out1, io["wq2"], io["wk2"], io["wv2"],
                      io["wo2"], bsb["bq2"], bsb["bk2"], bvbc["bv2"],
                      bsb["bo2"], False, bsb["g2"], bsb["be2"])

    # ---- FFN -------------------------------------------------------------
    fp = tc.alloc_tile_pool(name="ffn", bufs=1)
    fT = res_tile("fT")
    for g in range(8):
        wf1g = fp.tile([P, DC, 512], F32R, name="wf1g", tag="wf1g", bufs=2)
        nc.sync.dma_start(
            out=wf1g, in_=io["wf1"].bitcast(F32R)[:, ds(512 * g, 512)].rearrange(
                "(c p) n -> p c n", p=P))
        hTg = fp.tile([P, 4, R], F32R, name="hTg", tag="hTg", bufs=2)
        for fo in range(4):
            ps = psA.tile([P, R], F32, tag="proj")
            for kc in range(DC):
                nc.tensor.matmul(ps, (wf1g[:, kc, ds(P * fo, P)]),
                                 (out2[:, kc, :]),
                                 start=(kc == 0), stop=(kc == DC - 1))
            nc.scalar.activation(hTg[:, fo, :], ps, AF.Relu,
                                 bias=bf1_sb[:, 4 * g + fo:4 * g + fo + 1])
        wf2g = fp.tile([P, 4, D], F32R, name="wf2g", tag="wf2g", bufs=2)
        nc.sync.dma_start(
            out=wf2g, in_=io["wf2"].bitcast(F32R)[ds(512 * g, 512), :].rearrange(
                "(c p) n -> p c n", p=P))
        for do in range(DC):
            ps = psA.tile([P, R], F32, tag="proj")
            for fc in range(4):
                nc.tensor.matmul(ps, (wf2g[:, fc, ds(P * do, P)]),
                                 (hTg[:, fc, :]),
                                 start=(fc == 0), stop=(fc == 3))
            if g == 0:
                nc.scalar.add(fT[:, do, :], ps, bsb["bf2"][:, do:do + 1])
            else:
                nc.vector.tensor_add(fT[:, do, :], fT[:, do, :], ps)
    for do in range(DC):
        nc.vector.tensor_add(fT[:, do, :], fT[:, do, :], out2[:, do, :])

    def _emit_out(c, t, g, be):
        stg = fp.tile([P, R], F32, name="ostg", tag="ostg", bufs=2)
        nc.vector.tensor_scalar(stg, t, g, be, op0=ALU.mult, op1=ALU.add)
        nc.sync.dma_start(out=io["outT"][ds(P * c, P), :], in_=stg)
    layernorm(fp, fT, bsb["g3"], bsb["be3"], _emit_out)
    fp.release()

    persist.release()
    consts.release()
    dram.release()
    for p in (psR, psV, psS, psA):
        p.release()


def build_nc():
    nc = bacc.Bacc("TRN2", target_bir_lowering=False, debug=False)
    io = {}

    def inp(name, shape):
        io[name] = nc.dram_tensor(name, shape, F32, kind="ExternalInput").ap()

    inp("yT", [D, S])
    inp("xT", [D, S])
    inp("yTo", [D, R])
    inp("mask", [S, R])
    for n in WDD:
        inp(n, [D, D])
    inp("wf1", [D, F])
    inp("wf2", [F, D])
    for n in BDD + LNP + ["bf2"]:
        inp(n, [D])
    inp("bf1", [F])
    io["outT"] = nc.dram_tensor("outT", [D, R], F32,
                                kind="ExternalOutput").ap()
    with tile.TileContext(nc) as tc:
        _build_body(tc, io)
    nc.compile()
    return nc


_NC = None


def _get_nc():
    global _NC
    if _NC is None:
        _NC = build_nc()
    return _NC


def make_in_maps(inputs):
    gi = {k: np.ascontiguousarray(np.asarray(v, np.float32))
          for k, v in inputs.items()}
    yT = [np.ascontiguousarray(gi["y"][b].T) for b in range(B)]
    xT = [np.ascontiguousarray(gi["X"][b].T) for b in range(B)]
    shared = {n: gi[n] for n in WDD + BDD + LNP + ["wf1", "wf2", "bf1", "bf2"]}
    in_maps = []
    for c in range(NCORES):
        b, r0 = c // GP, (c % GP) * R
        mask = (np.arange(S)[:, None] <= (r0 + np.arange(R))[None, :])
        in_maps.append(dict(
            yT=yT[b], xT=xT[b],
            yTo=np.ascontiguousarray(gi["y"][b, r0:r0 + R].T),
            mask=np.ascontiguousarray(mask.astype(np.float32)),
            **shared))
    return in_maps


def kernel(**inputs):
    nc = _get_nc()
    in_maps = make_in_maps(inputs)
    res = run_bass_kernel_spmd(nc, in_maps, core_ids=list(range(NCORES)))
    out = np.empty((B, S, D), np.float32)
    for c in range(NCORES):
        out[c // GP, (c % GP) * R:(c % GP + 1) * R, :] = \
            res.results[c]["outT"].T
    return out
